# revision 48
# baseline (speedup 1.0000x reference)
"""
Trainium2 Bass kernel for nn_MF_MGCN (5-band 2-layer GCN + MLP head).

Single fused device launch (vs the 4-launch baseline, ~5.24s -> ~0.1-0.18s):
  * BatchNorm statistics are reduced across the 8 cores with on-device
    AllReduce collectives (3x tiny: 10, 25, and 256 floats), and the BN
    coefficient algebra runs on-device, so no host round trips remain.
  * The func adjacency ships packed as [114, NBLK, 19] uint8 with a
    per-graph scale folded exactly into x (~1.5MB/core instead of
    ~20MB/core dense block-diagonal); it is converted u8->bf16 on-device
    and the block-diagonal stationary tiles are assembled in SBUF by
    strided DMAs over a zeroed background.
  * Math identical to the baseline derivation: GCN1 on a 1-channel input
    is rank-1, so relu(BN1) @ W2 collapses onto (relu(z), relu(-z)) and
    GCN2 aggregates just 2 channels/band through one shared structural
    block-diagonal matrix.
  * The executable (jit of shard_map over 8 cores) is cached across
    calls, and all inputs are device_put asynchronously per-core while
    later cores are still packing, so the launch itself is mostly the
    ~90ms axon dispatch floor.

All compute-engine operands keep partition base 0 (hardware restricts
bases to {0,32,64}); any partition reshuffling goes through DMA.

If structural assumptions fail (shared struct pattern, bt1 == 0), a pure
numpy fallback reproduces the reference exactly.
"""

import sys

sys.path.insert(0, "/opt/trn_rl_repo")

import numpy as np
import ml_dtypes

BF16 = ml_dtypes.bfloat16

# Problem constants (hardcoded per task contract).
B = 32768
NN = 19
N = B * NN
BANDS = 5
EF, ES = 120, 60
EPS = 1e-5
NCORES = 8
G = B // NCORES           # graphs per core = 4096
SLOT = 6                  # graphs per 114-row block
P114 = SLOT * NN          # 114

_KERNEL_CACHE = {}
_RUNNER_CACHE = {}

# all f32 small-weight tensors ride in one packed input (fewer PJRT args);
# order defines the offsets used by both the builder and the host packer
SMALL_SPECS = [
    ("w1r", (BANDS, 32)), ("g1w", (BANDS, 32)), ("w2k", (BANDS, 4, 32)),
    ("cvec", (BANDS, 6)), ("w1s", (NN, 10, 128)), ("l1bv", (128, 3)),
    ("w2l", (128, 32)), ("l2b", (32, 1)), ("w3l", (32, 2)), ("l3b", (2, 1)),
]
SMALL_OFFS = {}
_o = 0
for _n, _s in SMALL_SPECS:
    SMALL_OFFS[_n] = _o
    _o += int(np.prod(_s))
SMALL_TOTAL = _o


# --------------------------------------------------------------------------
# numpy fallback (exact reference math)
# --------------------------------------------------------------------------
def _bn_np(h, g, b):
    m = h.mean(0)
    v = h.var(0)
    return (h - m) / np.sqrt(v + EPS) * g + b


def _gcn_np(h, W, b, src, dst, ew, n):
    h = h @ W
    deg = np.zeros(n, np.float64)
    np.add.at(deg, dst, ew)
    deg += 1.0
    dinv = 1.0 / np.sqrt(deg)
    norm = dinv[src] * ew * dinv[dst]
    agg = np.zeros_like(h, dtype=np.float64)
    np.add.at(agg, dst, norm[:, None] * h[src])
    return agg + (dinv * dinv)[:, None] * h + b


def _fallback_numpy(i):
    x = np.asarray(i["x"], np.float64)
    sf, df = np.asarray(i["edge_index_func"][0]), np.asarray(i["edge_index_func"][1])
    ss, ds = np.asarray(i["edge_index_struct"][0]), np.asarray(i["edge_index_struct"][1])
    ew = np.asarray(i["edge_weight_func"], np.float64)
    ews = np.ones(ss.shape[0], np.float64)
    n = x.shape[0]
    outs = []
    for b in range(BANDS):
        h = _gcn_np(x[:, b : b + 1], np.asarray(i["W1"][b], np.float64),
                    np.asarray(i["b1"][b], np.float64), sf, df, ew, n)
        h = np.maximum(_bn_np(h, np.asarray(i["g1"][b], np.float64),
                              np.asarray(i["bt1"][b], np.float64)), 0)
        h = _gcn_np(h, np.asarray(i["W2"][b], np.float64),
                    np.asarray(i["b2"][b], np.float64), ss, ds, ews, n)
        h = np.maximum(_bn_np(h, np.asarray(i["g2"][b], np.float64),
                              np.asarray(i["bt2"][b], np.float64)), 0)
        outs.append(h.reshape(n // NN, NN * 2))
    xc = np.concatenate(outs, axis=1)
    h = np.maximum(_bn_np(xc @ np.asarray(i["lin1_W"], np.float64)
                          + np.asarray(i["lin1_b"], np.float64),
                          np.asarray(i["g3"], np.float64),
                          np.asarray(i["bt3"], np.float64)), 0)
    h = np.maximum(h @ np.asarray(i["lin2_W"], np.float64)
                   + np.asarray(i["lin2_b"], np.float64), 0)
    out = h @ np.asarray(i["lin3_W"], np.float64) + np.asarray(i["lin3_b"], np.float64)
    return out.astype(np.float32)


# --------------------------------------------------------------------------
# fused Bass kernel builder
# --------------------------------------------------------------------------
def _build_fused(nblk, g_per_core, ncores, n_total, b_total):
    import concourse.bacc as bacc
    import concourse.mybir as mybir
    from concourse import tile

    f32, bf16, u8 = mybir.dt.float32, mybir.dt.bfloat16, mybir.dt.uint8
    f16 = mybir.dt.float16
    Relu = mybir.ActivationFunctionType.Relu
    Sqrt = mybir.ActivationFunctionType.Sqrt
    ALU = mybir.AluOpType
    AX = mybir.AxisListType

    nslot = nblk * SLOT
    npad = nslot - g_per_core          # pad graph slots (live in last block)
    assert 0 <= npad < SLOT
    CH1 = min(48, nblk)                # L1 blocks per chunk
    CH2 = min(51, nblk)                # L2 blocks per chunk (510 moving cols)
    rgroups = [list(range(ncores))]

    nc = bacc.Bacc(None, target_bir_lowering=False, num_devices=ncores)

    xb = nc.dram_tensor("xb", [P114, nblk, BANDS], bf16, kind="ExternalInput")
    afp = nc.dram_tensor("afp", [P114, nblk, NN], u8, kind="ExternalInput")
    asb = nc.dram_tensor("asb", [P114, 128], bf16, kind="ExternalInput")
    wpk = nc.dram_tensor("wpk", [1, SMALL_TOTAL], f32, kind="ExternalInput")
    yout = nc.dram_tensor("yout", [2, nslot], f16, kind="ExternalOutput")

    _sizes = {n: int(np.prod(s)) for n, s in SMALL_SPECS}

    def _wld(name, pattern, **kw):
        off = SMALL_OFFS[name]
        return wpk[:, off : off + _sizes[name]].rearrange(pattern, **kw)

    inv_n = 1.0 / float(n_total)
    inv_b = 1.0 / float(b_total)

    with tile.TileContext(nc) as tc:
        with (
            tc.tile_pool(name="const", bufs=1) as cp,
            tc.tile_pool(name="big", bufs=1) as bp,
            tc.tile_pool(name="scr", bufs=2) as sp,
            tc.tile_pool(name="scr1", bufs=1) as sp1,
            tc.tile_pool(name="dram", bufs=1, space="DRAM") as dp,
        ):
            # ---------- persistent small loads ----------
            as_t = cp.tile([P114, 128], bf16)
            nc.sync.dma_start(as_t[:], asb[:])
            w1r_t = cp.tile([BANDS, 32], f32)
            nc.sync.dma_start(w1r_t[:], _wld("w1r", "one (p c) -> p (one c)",
                                             p=BANDS))
            g1w_t = cp.tile([BANDS, 32], f32)
            nc.sync.dma_start(g1w_t[:], _wld("g1w", "one (p c) -> p (one c)",
                                             p=BANDS))
            w2k_t = cp.tile([BANDS, 4, 32], f32)
            nc.sync.dma_start(w2k_t[:], _wld("w2k", "one (p a b) -> p a (one b)",
                                             p=BANDS, a=4))
            cv_t = cp.tile([BANDS, 6], f32)
            nc.sync.dma_start(cv_t[:], _wld("cvec", "one (p c) -> p (one c)",
                                            p=BANDS))
            w1s_t = cp.tile([NN, 10, 128], f32)
            nc.sync.dma_start(w1s_t[:], _wld("w1s", "one (p a b) -> p a (one b)",
                                             p=NN, a=10))
            l1b_t = cp.tile([128, 3], f32)
            nc.sync.dma_start(l1b_t[:], _wld("l1bv", "one (p c) -> p (one c)",
                                             p=128))
            w2l_t = cp.tile([128, 32], f32)
            nc.sync.dma_start(w2l_t[:], _wld("w2l", "one (p c) -> p (one c)",
                                             p=128))
            l2b_t = cp.tile([32, 1], f32)
            nc.sync.dma_start(l2b_t[:], _wld("l2b", "one (p c) -> p (one c)",
                                             p=32))
            w3l_t = cp.tile([32, 2], f32)
            nc.sync.dma_start(w3l_t[:], _wld("w3l", "one (p c) -> p (one c)",
                                             p=32))
            l3b_t = cp.tile([2, 1], f32)
            nc.sync.dma_start(l3b_t[:], _wld("l3b", "one (p c) -> p (one c)",
                                             p=2))
            ones_t = cp.tile([128, 1], f32)
            nc.vector.memset(ones_t[:], 1.0)
            eps128 = cp.tile([128, 1], f32)
            nc.vector.memset(eps128[:], EPS)
            zpad = cp.tile([P114 - NN, 2 * BANDS], bf16)
            nc.vector.memset(zpad[:], 0.0)

            # DRAM bounce buffers for collectives
            cc1_in = dp.tile([10, 1], f32)
            cc1_out = dp.tile([1, 10], f32)
            cc2_in = dp.tile([25, 1], f32)
            cc2_out = dp.tile([1, 25], f32)
            cc3_in = dp.tile([128, 2], f32)
            cc3_out = dp.tile([128, 2], f32)
            abc_d = dp.tile([1, 30], f32)

            y1_t = bp.tile([128, nblk, SLOT], f32)

            with tc.tile_pool(name="pv", bufs=1) as pv:
                v_t = pv.tile([128, nblk, 2 * BANDS], f32)

                with tc.tile_pool(name="pu", bufs=1) as pu:
                    u_t = pu.tile([P114, nblk, 2 * BANDS], bf16)

                    # ---------- L1: s = AfT_blockdiag @ x ----------
                    with tc.tile_pool(name="p1", bufs=1) as p1:
                        x_t = p1.tile([P114, nblk, BANDS], bf16)
                        nc.sync.dma_start(x_t[:], xb[:])
                        s_t = p1.tile([128, nblk, BANDS], f32)
                        a0 = p1.tile([P114, CH1, 128], bf16)
                        a1 = p1.tile([P114, CH1, 128], bf16)
                        nc.vector.memset(a0[:], 0.0)
                        nc.vector.memset(a1[:], 0.0)
                        a_bufs = [a0, a1]
                        nch1 = (nblk + CH1 - 1) // CH1
                        with tc.tile_pool(name="ps1", bufs=4, space="PSUM") as pp1:
                            for c in range(nch1):
                                c0 = c * CH1
                                nb = min(CH1, nblk - c0)
                                a_t = a_bufs[c % 2]
                                # u8 chunk -> bf16 (per-graph scale is folded
                                # into xb on the host), then scatter into the
                                # block-diagonal positions
                                afq = sp.tile([P114, CH1, NN], u8, tag="afq")
                                nc.sync.dma_start(afq[:, :nb, :],
                                                  afp[:, c0 : c0 + nb, :])
                                afb = sp.tile([P114, CH1, NN], bf16, tag="afb")
                                nc.vector.tensor_copy(out=afb[:, :nb, :],
                                                      in_=afq[:, :nb, :])
                                for p in range(SLOT):
                                    nc.sync.dma_start(
                                        a_t[p * NN : (p + 1) * NN, :nb,
                                            p * NN : (p + 1) * NN],
                                        afb[p * NN : (p + 1) * NN, :nb, :],
                                    )
                                ps = pp1.tile([128, CH1, BANDS], f32, tag="ps1")
                                for j in range(nb):
                                    nc.tensor.matmul(
                                        ps[:, j, :], a_t[:, j, :], x_t[:, c0 + j, :],
                                        start=True, stop=True,
                                    )
                                nc.vector.tensor_copy(out=s_t[:, c0 : c0 + nb, :],
                                                      in_=ps[:, :nb, :])

                            # ---------- L1 stats: sum(s), sum(s^2) per band ----
                            part1 = cp.tile([128, 10], f32)
                            for b in range(BANDS):
                                nc.vector.tensor_reduce(
                                    out=part1[:, b : b + 1], in_=s_t[:, :, b],
                                    axis=AX.X, op=ALU.add)
                                scr = sp.tile([128, nblk], f32, tag="scr")
                                nc.vector.tensor_tensor(
                                    out=scr[:], in0=s_t[:, :, b],
                                    in1=s_t[:, :, b], op=ALU.mult)
                                nc.vector.tensor_reduce(
                                    out=part1[:, 5 + b : 6 + b], in_=scr[:],
                                    axis=AX.X, op=ALU.add)
                            pst1 = pp1.tile([10, 1], f32, tag="pst1")
                            nc.tensor.matmul(pst1[:], part1[:], ones_t[:],
                                             start=True, stop=True)
                            st1_t = cp.tile([10, 1], f32)
                            nc.vector.tensor_copy(out=st1_t[:], in_=pst1[:])

                        nc.sync.dma_start(cc1_in[:], st1_t[:])
                        nc.gpsimd.collective_compute(
                            "AllReduce", ALU.add, replica_groups=rgroups,
                            ins=[cc1_in[:].opt()], outs=[cc1_out[:].opt()],
                        )

                        # ---------- BN1 coefficients ----------
                        mv_t = cp.tile([BANDS, 2], f32)   # col0 mu1, col1 E[s^2]
                        nc.sync.dma_start(
                            mv_t[:, 0:1],
                            cc1_out[:, 0:BANDS].rearrange("one b -> b one"))
                        nc.sync.dma_start(
                            mv_t[:, 1:2],
                            cc1_out[:, BANDS : 2 * BANDS]
                            .rearrange("one b -> b one"))
                        nc.vector.tensor_scalar(out=mv_t[:], in0=mv_t[:],
                                                scalar1=inv_n, scalar2=None,
                                                op0=ALU.mult)
                        var5 = cp.tile([BANDS, 1], f32)
                        nc.vector.tensor_tensor(out=var5[:], in0=mv_t[:, 0:1],
                                                in1=mv_t[:, 0:1], op=ALU.mult)
                        nc.vector.tensor_tensor(out=var5[:], in0=mv_t[:, 1:2],
                                                in1=var5[:], op=ALU.subtract)
                        # a = w1r * rsqrt(var*w1r^2 + eps) * g1   [5, 32]
                        a5 = cp.tile([BANDS, 32], f32)
                        nc.vector.tensor_tensor(out=a5[:], in0=w1r_t[:],
                                                in1=w1r_t[:], op=ALU.mult)
                        nc.vector.tensor_scalar(out=a5[:], in0=a5[:],
                                                scalar1=var5[:, 0:1],
                                                scalar2=None, op0=ALU.mult)
                        nc.scalar.activation(a5[:], a5[:], Sqrt,
                                             bias=eps128[:BANDS, 0:1])
                        nc.vector.reciprocal(a5[:], a5[:])
                        nc.vector.tensor_tensor(out=a5[:], in0=a5[:],
                                                in1=w1r_t[:], op=ALU.mult)
                        nc.vector.tensor_tensor(out=a5[:], in0=a5[:],
                                                in1=g1w_t[:], op=ALU.mult)
                        # apm [5, 4, 32] cols (k, sign)
                        apm = cp.tile([BANDS, 4, 32], f32)
                        nc.vector.tensor_copy(out=apm[:, 0:1, :], in_=a5[:])
                        nc.vector.tensor_scalar(out=apm[:, 1:2, :], in0=a5[:],
                                                scalar1=-1.0, scalar2=None,
                                                op0=ALU.mult)
                        nc.vector.tensor_copy(out=apm[:, 2:3, :],
                                              in_=apm[:, 0:1, :])
                        nc.vector.tensor_copy(out=apm[:, 3:4, :],
                                              in_=apm[:, 1:2, :])
                        nc.scalar.activation(apm[:], apm[:], Relu)
                        # pq [5, 2, 2]: (band, k, sign)
                        prod = cp.tile([BANDS, 4, 32], f32)
                        nc.vector.tensor_tensor(out=prod[:], in0=w2k_t[:],
                                                in1=apm[:], op=ALU.mult)
                        pq = cp.tile([BANDS, 2, 2], f32)
                        nc.vector.tensor_reduce(out=pq[:], in_=prod[:],
                                                axis=AX.X, op=ALU.add)
                        p_ap = pq[:, :, 0:1]
                        q_ap = pq[:, :, 1:2]

                        # mu1 broadcast across partitions for z = s - mu
                        mu_bc = cp.tile([128, BANDS], f32)
                        nc.sync.dma_start(
                            mu_bc[:],
                            cc1_out[:, 0:BANDS].to_broadcast([128, BANDS]))
                        nc.vector.tensor_scalar(out=mu_bc[:], in0=mu_bc[:],
                                                scalar1=inv_n, scalar2=None,
                                                op0=ALU.mult)

                        # ---------- u = relu(+-(s - mu)) ----------
                        nc.vector.tensor_tensor(
                            out=u_t[:, :, 0:BANDS], in0=s_t[:P114],
                            in1=mu_bc[:P114, None, :]
                            .to_broadcast([P114, nblk, BANDS]),
                            op=ALU.subtract,
                        )
                        nc.vector.tensor_scalar(
                            out=u_t[:, :, BANDS : 2 * BANDS],
                            in0=u_t[:, :, 0:BANDS], scalar1=-1.0,
                            scalar2=None, op0=ALU.mult)
                        nc.scalar.activation(u_t[:], u_t[:], Relu)
                        if npad:
                            # zero pad-slot rows via DMA (engine partition
                            # bases are restricted to {0,32,64})
                            nc.sync.dma_start(
                                u_t[(SLOT - npad) * NN : P114, nblk - 1, :],
                                zpad[: npad * NN, :])
                    # p1 closed: x_t / a / s_t freed

                    # ---------- L2: v = As_blockdiag @ u ----------
                    nch2 = (nblk + CH2 - 1) // CH2
                    with tc.tile_pool(name="ps2", bufs=4, space="PSUM") as pp2:
                        for c in range(nch2):
                            c0 = c * CH2
                            nb = min(CH2, nblk - c0)
                            ps = pp2.tile([128, CH2, 2 * BANDS], f32, tag="ps2")
                            nc.tensor.matmul(
                                ps[:, :nb, :], as_t[:], u_t[:, c0 : c0 + nb, :],
                                start=True, stop=True,
                            )
                            nc.vector.tensor_copy(out=v_t[:, c0 : c0 + nb, :],
                                                  in_=ps[:, :nb, :])

                        # ---------- L2 stats ----------
                        part2 = cp.tile([128, 25], f32)
                        for b in range(BANDS):
                            nc.vector.tensor_reduce(
                                out=part2[:, b : b + 1], in_=v_t[:, :, b],
                                axis=AX.X, op=ALU.add)
                            nc.vector.tensor_reduce(
                                out=part2[:, 5 + b : 6 + b],
                                in_=v_t[:, :, 5 + b], axis=AX.X, op=ALU.add)
                            for k, (i0, i1) in ((10, (b, b)),
                                                (15, (5 + b, 5 + b)),
                                                (20, (b, 5 + b))):
                                scr = sp.tile([128, nblk], f32, tag="scr")
                                nc.vector.tensor_tensor(
                                    out=scr[:], in0=v_t[:, :, i0],
                                    in1=v_t[:, :, i1], op=ALU.mult)
                                nc.vector.tensor_reduce(
                                    out=part2[:, k + b : k + b + 1],
                                    in_=scr[:], axis=AX.X, op=ALU.add)
                        pst2 = pp2.tile([25, 1], f32, tag="pst2")
                        nc.tensor.matmul(pst2[:], part2[:], ones_t[:],
                                         start=True, stop=True)
                        st2_t = cp.tile([25, 1], f32)
                        nc.vector.tensor_copy(out=st2_t[:], in_=pst2[:])

                    nc.sync.dma_start(cc2_in[:], st2_t[:])
                    nc.gpsimd.collective_compute(
                        "AllReduce", ALU.add, replica_groups=rgroups,
                        ins=[cc2_in[:].opt()], outs=[cc2_out[:].opt()],
                    )
                # pu closed: u_t freed

                # ---------- BN2 coefficients: A,B,C [5, 2(k)] ----------
                stm5 = cp.tile([BANDS, 5], f32)
                for gidx in range(5):
                    nc.sync.dma_start(
                        stm5[:, gidx : gidx + 1],
                        cc2_out[:, gidx * BANDS : (gidx + 1) * BANDS]
                        .rearrange("one b -> b one"))
                nc.vector.tensor_scalar(out=stm5[:], in0=stm5[:], scalar1=inv_n,
                                        scalar2=None, op0=ALU.mult)
                mom = cp.tile([BANDS, 3], f32)
                nc.vector.tensor_tensor(out=mom[:, 0:1], in0=stm5[:, 0:1],
                                        in1=stm5[:, 0:1], op=ALU.mult)
                nc.vector.tensor_tensor(out=mom[:, 0:1], in0=stm5[:, 2:3],
                                        in1=mom[:, 0:1], op=ALU.subtract)
                nc.vector.tensor_tensor(out=mom[:, 1:2], in0=stm5[:, 1:2],
                                        in1=stm5[:, 1:2], op=ALU.mult)
                nc.vector.tensor_tensor(out=mom[:, 1:2], in0=stm5[:, 3:4],
                                        in1=mom[:, 1:2], op=ALU.subtract)
                nc.vector.tensor_tensor(out=mom[:, 2:3], in0=stm5[:, 0:1],
                                        in1=stm5[:, 1:2], op=ALU.mult)
                nc.vector.tensor_tensor(out=mom[:, 2:3], in0=stm5[:, 4:5],
                                        in1=mom[:, 2:3], op=ALU.subtract)
                mu2 = cp.tile([BANDS, 2], f32)
                t2a = cp.tile([BANDS, 2], f32)
                nc.vector.tensor_scalar(out=mu2[:], in0=p_ap,
                                        scalar1=stm5[:, 0:1],
                                        scalar2=None, op0=ALU.mult)
                nc.vector.tensor_scalar(out=t2a[:], in0=q_ap,
                                        scalar1=stm5[:, 1:2],
                                        scalar2=None, op0=ALU.mult)
                nc.vector.tensor_tensor(out=mu2[:], in0=mu2[:], in1=t2a[:],
                                        op=ALU.add)
                nc.vector.tensor_tensor(out=mu2[:], in0=mu2[:], in1=cv_t[:, 0:2],
                                        op=ALU.add)
                var2 = cp.tile([BANDS, 2], f32)
                nc.vector.tensor_tensor(out=var2[:], in0=p_ap, in1=p_ap,
                                        op=ALU.mult)
                nc.vector.tensor_scalar(out=var2[:], in0=var2[:],
                                        scalar1=mom[:, 0:1],
                                        scalar2=None, op0=ALU.mult)
                nc.vector.tensor_tensor(out=t2a[:], in0=q_ap, in1=q_ap,
                                        op=ALU.mult)
                nc.vector.tensor_scalar(out=t2a[:], in0=t2a[:],
                                        scalar1=mom[:, 1:2],
                                        scalar2=None, op0=ALU.mult)
                nc.vector.tensor_tensor(out=var2[:], in0=var2[:], in1=t2a[:],
                                        op=ALU.add)
                nc.vector.tensor_tensor(out=t2a[:], in0=p_ap, in1=q_ap,
                                        op=ALU.mult)
                nc.vector.tensor_scalar(out=t2a[:], in0=t2a[:],
                                        scalar1=mom[:, 2:3],
                                        scalar2=None, op0=ALU.mult)
                nc.vector.tensor_scalar(out=t2a[:], in0=t2a[:], scalar1=2.0,
                                        scalar2=None, op0=ALU.mult)
                nc.vector.tensor_tensor(out=var2[:], in0=var2[:], in1=t2a[:],
                                        op=ALU.add)
                rs2 = cp.tile([BANDS, 2], f32)
                nc.scalar.activation(rs2[:], var2[:], Sqrt,
                                     bias=eps128[:BANDS, 0:1])
                nc.vector.reciprocal(rs2[:], rs2[:])
                nc.vector.tensor_tensor(out=rs2[:], in0=rs2[:], in1=cv_t[:, 2:4],
                                        op=ALU.mult)   # rsqrt(var+eps)*g2
                abc = cp.tile([BANDS, 6], f32)
                nc.vector.tensor_tensor(out=abc[:, 0:2], in0=p_ap, in1=rs2[:],
                                        op=ALU.mult)
                nc.vector.tensor_tensor(out=abc[:, 2:4], in0=q_ap, in1=rs2[:],
                                        op=ALU.mult)
                nc.vector.tensor_tensor(out=t2a[:], in0=cv_t[:, 0:2], in1=mu2[:],
                                        op=ALU.subtract)
                nc.vector.tensor_tensor(out=t2a[:], in0=t2a[:], in1=rs2[:],
                                        op=ALU.mult)
                nc.vector.tensor_tensor(out=abc[:, 4:6], in0=t2a[:],
                                        in1=cv_t[:, 4:6], op=ALU.add)
                for ci in range(3):
                    nc.sync.dma_start(
                        abc_d[:, ci * 10 : (ci + 1) * 10]
                        .rearrange("one (k b) -> b (one k)", k=2),
                        abc[:, 2 * ci : 2 * ci + 2])
                coef_bc = cp.tile([128, 30], f32)
                nc.sync.dma_start(coef_bc[:], abc_d[:].to_broadcast([128, 30]))

                # ---------- L3: xc = relu(A*v+ + B*v- + C); y1 = lin1(xc) ----
                with (
                    tc.tile_pool(name="p3", bufs=2) as p3,
                    tc.tile_pool(name="ps3", bufs=4, space="PSUM") as pp3,
                ):
                    for s in range(SLOT):
                        vs = p3.tile([NN, nblk, 2 * BANDS], f32, tag="vs")
                        nc.sync.dma_start(vs[:], v_t[s * NN : (s + 1) * NN, :, :])
                        xcs = p3.tile([NN, nblk, 2 * BANDS], f32, tag="xcs")
                        for k in range(2):
                            ksl = slice(k * BANDS, (k + 1) * BANDS)
                            scrb = sp1.tile([NN, nblk, BANDS], f32, tag="scrb")
                            nc.vector.tensor_tensor(
                                out=xcs[:, :, ksl], in0=vs[:, :, 0:BANDS],
                                in1=coef_bc[:NN, None,
                                            k * BANDS : (k + 1) * BANDS]
                                .to_broadcast([NN, nblk, BANDS]),
                                op=ALU.mult,
                            )
                            nc.vector.tensor_tensor(
                                out=scrb[:], in0=vs[:, :, BANDS : 2 * BANDS],
                                in1=coef_bc[:NN, None,
                                            10 + k * BANDS : 10 + (k + 1) * BANDS]
                                .to_broadcast([NN, nblk, BANDS]),
                                op=ALU.mult,
                            )
                            nc.vector.tensor_tensor(out=xcs[:, :, ksl],
                                                    in0=xcs[:, :, ksl],
                                                    in1=scrb[:], op=ALU.add)
                            nc.vector.tensor_tensor(
                                out=xcs[:, :, ksl], in0=xcs[:, :, ksl],
                                in1=coef_bc[:NN, None,
                                            20 + k * BANDS : 20 + (k + 1) * BANDS]
                                .to_broadcast([NN, nblk, BANDS]),
                                op=ALU.add,
                            )
                        nc.scalar.activation(xcs[:], xcs[:], Relu)
                        for c0 in range(0, nblk, 512):
                            nb = min(512, nblk - c0)
                            ps = pp3.tile([128, 512], f32, tag="ps3")
                            for j in range(10):
                                nc.tensor.matmul(
                                    ps[:, :nb], w1s_t[:, j, :],
                                    xcs[:, c0 : c0 + nb, j],
                                    start=(j == 0), stop=(j == 9),
                                )
                            nc.vector.tensor_scalar(
                                out=y1_t[:, c0 : c0 + nb, s], in0=ps[:, :nb],
                                scalar1=l1b_t[:, 0:1], scalar2=None,
                                op0=ALU.add)
            # pv closed: v_t freed
            if npad:
                nc.vector.memset(y1_t[:, nblk - 1, SLOT - npad : SLOT], 0.0)

            # ---------- BN3 stats + head ----------
            with tc.tile_pool(name="ph", bufs=1) as ph:
                part3 = cp.tile([128, 2], f32)
                nc.vector.tensor_reduce(out=part3[:, 0:1], in_=y1_t[:],
                                        axis=AX.XY, op=ALU.add)
                x2_t = ph.tile([128, nblk, SLOT], f32)
                nc.vector.tensor_tensor(out=x2_t[:], in0=y1_t[:], in1=y1_t[:],
                                        op=ALU.mult)
                nc.vector.tensor_reduce(out=part3[:, 1:2], in_=x2_t[:],
                                        axis=AX.XY, op=ALU.add)
                nc.sync.dma_start(cc3_in[:], part3[:])
                nc.gpsimd.collective_compute(
                    "AllReduce", ALU.add, replica_groups=rgroups,
                    ins=[cc3_in[:].opt()], outs=[cc3_out[:].opt()],
                )

                st3_t = cp.tile([128, 2], f32)
                nc.sync.dma_start(st3_t[:], cc3_out[:])
                nc.vector.tensor_scalar(out=st3_t[:], in0=st3_t[:],
                                        scalar1=inv_b, scalar2=None,
                                        op0=ALU.mult)
                g3c = cp.tile([128, 2], f32)        # col0 G3, col1 B3
                nc.vector.tensor_tensor(out=g3c[:, 0:1], in0=st3_t[:, 0:1],
                                        in1=st3_t[:, 0:1], op=ALU.mult)
                nc.vector.tensor_tensor(out=g3c[:, 0:1], in0=st3_t[:, 1:2],
                                        in1=g3c[:, 0:1], op=ALU.subtract)
                nc.scalar.activation(g3c[:, 0:1], g3c[:, 0:1], Sqrt,
                                     bias=eps128[:, 0:1])
                nc.vector.reciprocal(g3c[:, 0:1], g3c[:, 0:1])
                nc.vector.tensor_tensor(out=g3c[:, 0:1], in0=g3c[:, 0:1],
                                        in1=l1b_t[:, 1:2], op=ALU.mult)
                nc.vector.tensor_tensor(out=g3c[:, 1:2], in0=st3_t[:, 0:1],
                                        in1=g3c[:, 0:1], op=ALU.mult)
                nc.vector.tensor_tensor(out=g3c[:, 1:2], in0=l1b_t[:, 2:3],
                                        in1=g3c[:, 1:2], op=ALU.subtract)
                nc.vector.tensor_scalar(out=x2_t[:], in0=y1_t[:],
                                        scalar1=g3c[:, 0:1], scalar2=None,
                                        op0=ALU.mult)
                nc.scalar.activation(x2_t[:], x2_t[:], Relu, bias=g3c[:, 1:2])

                x3_t = ph.tile([32, nslot], f32)
                x2f = x2_t[:].rearrange("p a b -> p (a b)")
                with tc.tile_pool(name="ps4", bufs=4, space="PSUM") as pp4:
                    for c0 in range(0, nslot, 512):
                        nb = min(512, nslot - c0)
                        ps4 = pp4.tile([32, 512], f32, tag="ps4")
                        nc.tensor.matmul(ps4[:, :nb], w2l_t[:],
                                         x2f[:, c0 : c0 + nb],
                                         start=True, stop=True)
                        nc.scalar.activation(x3_t[:, c0 : c0 + nb], ps4[:, :nb],
                                             Relu, bias=l2b_t[:, 0:1])
                    for c0 in range(0, nslot, 512):
                        nb = min(512, nslot - c0)
                        ps5 = pp4.tile([2, 512], f32, tag="ps5")
                        nc.tensor.matmul(ps5[:, :nb], w3l_t[:],
                                         x3_t[:, c0 : c0 + nb],
                                         start=True, stop=True)
                        yos = sp.tile([2, 512], f16, tag="yos")
                        nc.vector.tensor_scalar(out=yos[:, :nb], in0=ps5[:, :nb],
                                                scalar1=l3b_t[:, 0:1],
                                                scalar2=None, op0=ALU.add)
                        nc.sync.dma_start(yout[:, c0 : c0 + nb], yos[:, :nb])

    nc.compile()
    return nc


def _get_kernel():
    if "k" not in _KERNEL_CACHE:
        _KERNEL_CACHE["k"] = _build_fused((G + SLOT - 1) // SLOT,
                                          G, NCORES, N, B)
    return _KERNEL_CACHE["k"]


# --------------------------------------------------------------------------
# cached-jit runner (mirrors bass2jax.run_bass_via_pjrt, but caches the
# traced/compiled executable across calls)
# --------------------------------------------------------------------------
def _get_runner(nc, n_cores):
    key = id(nc)
    if key in _RUNNER_CACHE:
        return _RUNNER_CACHE[key]

    import jax
    from jax.experimental.shard_map import shard_map
    from jax.sharding import Mesh, PartitionSpec
    from concourse import bass2jax, mybir

    bass2jax.install_neuronx_cc_hook()
    assert nc.dbg_addr is None
    partition_name = nc.partition_id_tensor.name if nc.partition_id_tensor else None

    in_names, out_names, out_avals, zero_shapes = [], [], [], []
    for alloc in nc.m.functions[0].allocations:
        if not isinstance(alloc, mybir.MemoryLocationSet):
            continue
        name = alloc.memorylocations[0].name
        if alloc.kind == "ExternalInput":
            if name != partition_name:
                in_names.append(name)
        elif alloc.kind == "ExternalOutput":
            out_names.append(name)
            shape = tuple(alloc.tensor_shape)
            dtype = mybir.dt.np(alloc.dtype)
            out_avals.append(jax.core.ShapedArray(shape, dtype))
            zero_shapes.append((shape, dtype))
    n_params = len(in_names)
    all_in_names = (in_names + out_names
                    + ([partition_name] if partition_name else []))
    donate = tuple(range(n_params, n_params + len(out_names)))

    def _body(*args):
        operands = list(args)
        if partition_name is not None:
            operands.append(bass2jax.partition_id_tensor())
        outs = bass2jax._bass_exec_p.bind(
            *operands,
            out_avals=tuple(out_avals),
            in_names=tuple(all_in_names),
            out_names=tuple(out_names),
            lowering_input_output_aliases=(),
            sim_require_finite=True,
            sim_require_nnan=True,
            nc=nc,
        )
        return tuple(outs)

    devices = jax.devices()[:n_cores]
    assert len(devices) == n_cores
    mesh = Mesh(np.asarray(devices), ("core",))
    in_specs = (PartitionSpec("core"),) * (n_params + len(out_names))
    out_specs = (PartitionSpec("core"),) * len(out_names)
    sharded = jax.jit(
        shard_map(_body, mesh=mesh, in_specs=in_specs, out_specs=out_specs,
                  check_rep=False),
        donate_argnums=donate, keep_unused=True,
    )
    runner = (sharded, in_names, out_names, zero_shapes)
    _RUNNER_CACHE[key] = runner
    return runner


def _get_sharding(n_cores):
    key = ("sharding", n_cores)
    if key not in _RUNNER_CACHE:
        import jax
        from jax.sharding import Mesh, PartitionSpec, NamedSharding

        mesh = Mesh(np.asarray(jax.devices()[:n_cores]), ("core",))
        _RUNNER_CACHE[key] = NamedSharding(mesh, PartitionSpec("core"))
    return _RUNNER_CACHE[key]


def _run(nc, in_maps, tag):
    """in_maps: list of per-core dicts (numpy), or a single dict of
    already-concatenated global arrays (numpy or device-resident)."""
    n_cores = NCORES if isinstance(in_maps, dict) else len(in_maps)
    sharded, in_names, out_names, zero_shapes = _get_runner(nc, n_cores)
    if isinstance(in_maps, dict):
        concat_in = [in_maps[name] for name in in_names]
        concat_zeros = in_maps.get("__zeros__")
    else:
        concat_in = [
            np.concatenate([np.asarray(in_maps[c][name])
                            for c in range(n_cores)], axis=0)
            for name in in_names
        ]
        concat_zeros = None
    if concat_zeros is None:
        concat_zeros = [
            np.zeros((n_cores * shape[0], *shape[1:]), dtype)
            for shape, dtype in zero_shapes
        ]
    out_arrs = sharded(*concat_in, *concat_zeros)
    try:
        # start the d2h copy while we wait for completion, so the output
        # payload streams inside the same round trip
        for a in out_arrs:
            a.copy_to_host_async()
    except Exception:
        pass
    return [
        {
            name: np.asarray(out_arrs[i]).reshape(
                n_cores, *zero_shapes[i][0])[c]
            for i, name in enumerate(out_names)
        }
        for c in range(n_cores)
    ]


# --------------------------------------------------------------------------
# main entry
# --------------------------------------------------------------------------
def kernel(**inputs) -> np.ndarray:
    x = np.asarray(inputs["x"], np.float32)
    eif = np.asarray(inputs["edge_index_func"])
    eis = np.asarray(inputs["edge_index_struct"])
    ew = np.asarray(inputs["edge_weight_func"], np.float32)
    W1 = np.asarray(inputs["W1"], np.float32)
    g1 = np.asarray(inputs["g1"], np.float32)
    bt1 = np.asarray(inputs["bt1"], np.float32)
    W2 = np.asarray(inputs["W2"], np.float32)
    b2 = np.asarray(inputs["b2"], np.float32)
    g2 = np.asarray(inputs["g2"], np.float32)
    bt2 = np.asarray(inputs["bt2"], np.float32)
    lin1_W = np.asarray(inputs["lin1_W"], np.float32)
    lin1_b = np.asarray(inputs["lin1_b"], np.float32)
    g3 = np.asarray(inputs["g3"], np.float32)
    bt3 = np.asarray(inputs["bt3"], np.float32)
    lin2_W = np.asarray(inputs["lin2_W"], np.float32)
    lin2_b = np.asarray(inputs["lin2_b"], np.float32)
    lin3_W = np.asarray(inputs["lin3_W"], np.float32)
    lin3_b = np.asarray(inputs["lin3_b"], np.float32)

    sf, df = eif[0].astype(np.int64), eif[1].astype(np.int64)
    ss, ds = eis[0].astype(np.int64), eis[1].astype(np.int64)

    # --- structural-assumption checks (else exact numpy fallback) ---
    gs = ss // NN
    ok = np.array_equal(gs, ds // NN) and np.array_equal(
        gs, np.repeat(np.arange(B), ES)
    )
    gf = sf // NN
    ok = ok and np.array_equal(gf, df // NN) and np.array_equal(
        gf, np.repeat(np.arange(B), EF)
    )
    ssl, dsl = ss % NN, ds % NN
    ok = ok and np.array_equal(ssl.reshape(B, ES), np.broadcast_to(ssl[:ES], (B, ES)))
    ok = ok and np.array_equal(dsl.reshape(B, ES), np.broadcast_to(dsl[:ES], (B, ES)))
    ok = ok and np.abs(bt1).max() == 0.0
    if not ok:
        return _fallback_numpy(inputs)

    try:
        return _device_pipeline(x, sf, df, ew, ssl, dsl, W1, g1, W2, b2, g2, bt2,
                                lin1_W, lin1_b, g3, bt3, lin2_W, lin2_b,
                                lin3_W, lin3_b)
    except Exception as e:
        import traceback
        print(f"device pipeline failed ({e}); numpy fallback", file=sys.stderr)
        traceback.print_exc()
        return _fallback_numpy(inputs)


def _pack_block_major(arr_g, ncols, NBLK, NSLOT, dtype=BF16):
    """[G, NN, ncols] -> [P114, NBLK, ncols] slot-major packing (zero pads)."""
    buf = np.zeros((NSLOT, NN, ncols), arr_g.dtype)
    buf[:G] = arr_g
    return np.ascontiguousarray(
        buf.reshape(NBLK, SLOT, NN, ncols).transpose(1, 2, 0, 3)
        .reshape(P114, NBLK, ncols)).astype(dtype)


def _asb_host(ssl, dsl):
    """Shared structural block-diagonal matrix (identical for all graphs)."""
    s0, d0 = ssl[:ES], dsl[:ES]
    deg_s = np.bincount(d0, minlength=NN).astype(np.float64) + 1.0
    dinv_s = 1.0 / np.sqrt(deg_s)
    AsT = np.zeros((NN, NN), np.float64)
    np.add.at(AsT, (s0, d0), dinv_s[s0] * dinv_s[d0])
    AsT[np.arange(NN), np.arange(NN)] += dinv_s * dinv_s
    asb = np.zeros((P114, 128), np.float32)
    for p in range(SLOT):
        asb[p * NN : (p + 1) * NN, p * NN : (p + 1) * NN] = AsT
    return asb.astype(BF16)


def _aft_core(sf, df, ew, ngraphs):
    """Normalized func adjacency for one core's graphs (local ids),
    transposed, self-loop folded: [ngraphs, 19src, 19dst]."""
    nloc = ngraphs * NN
    deg_f = np.bincount(df, weights=ew.astype(np.float64), minlength=nloc) + 1.0
    dinv_f = (1.0 / np.sqrt(deg_f)).astype(np.float32)
    norm_f = dinv_f[sf] * ew * dinv_f[df]
    gf = sf // NN
    idx = gf * (NN * NN) + (sf % NN) * NN + (df % NN)
    AfT = np.bincount(idx, weights=norm_f.astype(np.float64),
                      minlength=ngraphs * NN * NN
                      ).astype(np.float32).reshape(ngraphs, NN, NN)
    AfT[:, np.arange(NN), np.arange(NN)] += (dinv_f * dinv_f).reshape(ngraphs, NN)
    return AfT


def _small_weights(W1, g1, W2, b2, g2, bt2, lin1_W, lin1_b, g3, bt3,
                   lin2_W, lin2_b, lin3_W, lin3_b):
    w1r = np.ascontiguousarray(W1[:, 0, :])                    # [5, 32]
    # w2k [5, 4, 32]: col (k, sign) -> W2[b, :, k] (same for both signs)
    w2kk = np.empty((BANDS, 4, 32), np.float32)
    for k in range(2):
        for sgn in range(2):
            w2kk[:, k * 2 + sgn, :] = W2[:, :, k]
    cvec = np.ascontiguousarray(
        np.concatenate([b2, g2, bt2], axis=1))                 # [5, 6]
    # lin1 stationary per j=(k,b): w1s[n, j, :] = lin1_W[b*38 + n*2 + k]
    w1s = np.empty((NN, 10, 128), np.float32)
    for k in range(2):
        for b in range(BANDS):
            j = k * BANDS + b
            w1s[:, j, :] = lin1_W[b * 2 * NN + np.arange(NN) * 2 + k]
    l1bv = np.ascontiguousarray(
        np.stack([lin1_b, g3, bt3], axis=1))                   # [128, 3]
    vals = {
        "w1r": w1r, "g1w": g1, "w2k": w2kk, "cvec": cvec, "w1s": w1s,
        "l1bv": l1bv, "w2l": lin2_W, "l2b": lin2_b.reshape(32, 1),
        "w3l": lin3_W, "l3b": lin3_b.reshape(2, 1),
    }
    wpk = np.empty((1, SMALL_TOTAL), np.float32)
    for n, s in SMALL_SPECS:
        size = int(np.prod(s))
        wpk[0, SMALL_OFFS[n] : SMALL_OFFS[n] + size] = (
            np.ascontiguousarray(vals[n], dtype=np.float32).ravel())
    return wpk


def _device_pipeline(x, sf, df, ew, ssl, dsl, W1, g1, W2, b2, g2, bt2,
                     lin1_W, lin1_b, g3, bt3, lin2_W, lin2_b, lin3_W, lin3_b):
    import jax

    NBLK = (G + SLOT - 1) // SLOT
    NSLOT = NBLK * SLOT
    nc = _get_kernel()
    sh = _get_sharding(NCORES)
    devs = list(sh.mesh.devices.reshape(-1))

    # Big inputs: per core, compute adjacency -> quantize -> pack (pure
    # numpy, parallelized across cores in threads) -> start the async
    # transfer from the main thread as each core's pack completes, so the
    # link is busy while later cores still pack. The func adjacency ships
    # as uint8 with a per-graph scale folded exactly into xb
    # (s = q @ (x * scale_g)); edges are per-core contiguous (validated
    # by the structural check in kernel()).
    from concurrent.futures import ThreadPoolExecutor

    ef = sf.size // B
    x3 = x.reshape(B, NN, BANDS)

    def _prep_core(c):
        e0, e1 = c * G * ef, (c + 1) * G * ef
        base = c * G * NN
        AfT_c = _aft_core(sf[e0:e1] - base, df[e0:e1] - base, ew[e0:e1], G)
        neg = bool(AfT_c.min() < 0.0)
        scale_c = (AfT_c.max(axis=(1, 2)) / 255.0).astype(np.float32)  # > 0
        q = np.rint(AfT_c / scale_c[:, None, None]).astype(np.uint8)
        afp_c = _pack_block_major(q, NN, NBLK, NSLOT, dtype=np.uint8)
        xb_c = _pack_block_major(
            x3[c * G : (c + 1) * G] * scale_c[:, None, None],
            BANDS, NBLK, NSLOT)
        return afp_c, xb_c, neg

    afp_shards, xb_shards = [], []
    with ThreadPoolExecutor(max_workers=2) as ex:
        futs = [ex.submit(_prep_core, c) for c in range(NCORES)]
        for c, fut in enumerate(futs):
            afp_c, xb_c, neg = fut.result()
            if neg:
                # u8 quantization assumes non-negative entries (ew >= 0)
                raise ValueError("negative adjacency entries; using fallback")
            afp_shards.append(jax.device_put(afp_c, devs[c]))
            xb_shards.append(jax.device_put(xb_c, devs[c]))
    afp_d = jax.make_array_from_single_device_arrays(
        (NCORES * P114, NBLK, NN), sh, afp_shards)
    xb_d = jax.make_array_from_single_device_arrays(
        (NCORES * P114, NBLK, BANDS), sh, xb_shards)

    small = {"asb": _asb_host(ssl, dsl),
             "wpk": _small_weights(W1, g1, W2, b2, g2, bt2, lin1_W, lin1_b, g3,
                                   bt3, lin2_W, lin2_b, lin3_W, lin3_b)}
    global_map = {"afp": afp_d, "xb": xb_d}
    for name, arr in small.items():
        global_map[name] = jax.device_put(
            np.concatenate([arr] * NCORES, axis=0), sh)
    # pre-stage the donated output zero-buffers as well
    _, _, _, zero_shapes = _get_runner(nc, NCORES)

    def _zeros():
        return [jax.device_put(np.zeros((NCORES * s[0], *s[1:]), d), sh)
                for s, d in zero_shapes]

    if "warm" not in _KERNEL_CACHE:
        # absorb one-time post-compile warmup (executable + fetch-path
        # caches) into the first call so later calls run at steady state
        global_map["__zeros__"] = _zeros()
        _run(nc, global_map, "warmup")
        _KERNEL_CACHE["warm"] = True
    global_map["__zeros__"] = _zeros()
    # with threaded prep the packing may finish before the transfers do;
    # wait for them here so the launch itself stays a single round trip.
    # One block suffices: per-device transfer queues are ordered, and the
    # zeros shards were issued last on every device.
    global_map["__zeros__"][-1].block_until_ready()
    # keep the launch window free of GC pauses (the metric is one RTT;
    # a collection during result deserialization stretches it)
    import gc
    gc.collect()

    res = _run(nc, global_map, "fused")

    out = np.empty((B, 2), np.float32)
    for c in range(NCORES):
        out[c * G : (c + 1) * G] = res[c]["yout"][:, :G].T.astype(np.float32)
    return out
